# revision 27
# baseline (speedup 1.0000x reference)
"""Trainium2 Bass kernel for Mistral sliding-window attention (B=1, S=4096,
HID=1024, H=8 q-heads, KVH=2 kv-heads, D=128, WINDOW=2048).

Sequence-sharded across 8 NeuronCores (512 queries per core); each core
recomputes K/V for its 2560-key window.

Precision plan: fp8(e4m3) DoubleRow matmuls for the bulk work — K/V
projections of the 2048-key history (blocks 0-3) and P@V / denominator for
key chunks 0-15 — and f16 for the numerically sensitive edge: the core's own
512 rows (block 4: K/V/Q projections), P for key chunks 16-19, ctx, and
o_proj. Core 0's earliest queries attend only chunk-16+ keys (tiny softmax
windows, no error averaging), so their whole path stays f16; all other
regions average fp8 noise over >=512 keys.

wk8/wv8 are pre-scaled by 64 on the host (fp8 subnormal avoidance); the
device rescales V by 1/64 when writing v8 and folds the K-side 64 into the
exp scale of chunks 0-15.
"""

import sys
import numpy as np
from contextlib import ExitStack

if "/opt/trn_rl_repo" not in sys.path:
    sys.path.insert(0, "/opt/trn_rl_repo")

# ---------------------------------------------------------------- constants
FULL_CFG = dict(
    R=512,        # queries per core
    W=2048,       # sliding window
    HID=1024,     # hidden size
    H=8,          # query heads
    KVH=2,        # kv heads
    D=128,        # head dim
    THETA=10000.0,
    NCORES=8,
    WSCALE=64.0,  # host fp8 weight pre-scale
)


def _derived(cfg):
    R, W, HID = cfg["R"], cfg["W"], cfg["HID"]
    KVW = W + R
    HC = HID // 128
    NKC = KVW // 128
    assert W >= R and R <= 512 and HID % 128 == 0 and KVW % 128 == 0
    return KVW, HC, NKC


def build_program(cfg):
    import concourse.bass as bass
    import concourse.tile as tile
    from concourse import bacc, mybir

    f32, f16, f8 = mybir.dt.float32, mybir.dt.float16, mybir.dt.float8e4
    DR = mybir.MatmulPerfMode.DoubleRow
    Exp = mybir.ActivationFunctionType.Exp
    ts = bass.ts
    R, W, HID, H, KVH, D = (cfg["R"], cfg["W"], cfg["HID"], cfg["H"],
                            cfg["KVH"], cfg["D"])
    WS = cfg["WSCALE"]
    KVW, HC, NKC = _derived(cfg)
    GROUPS = H // KVH
    NPAIR = HC // 2
    HIE = W // 128              # first upper-edge (own-rows) chunk = 16
    ESC_LO = 1.0 / (np.sqrt(D) * WS)    # fp8 chunks: k carries x64
    ESC_HI = 1.0 / np.sqrt(D)           # f16 chunks

    nc = bacc.Bacc("TRN2", target_bir_lowering=False, debug=False)

    def din(name, shape, dt):
        return nc.dram_tensor(name, shape, dt, kind="ExternalInput").ap()

    xT8 = din("xT8", [128 * HC * W], f8)          # blocks 0-3, block-major
    xT16d = din("xT16", [128, HC, R], f16)        # block 4 (own rows)
    wk8 = din("wk8", [128, HC * KVH * D], f8)
    wv8 = din("wv8", [128, HC * KVH * D], f8)
    wk16d = din("wk16", [128, HC * KVH * D], f16)
    wv16d = din("wv16", [128, HC * KVH * D], f16)
    wq16d = din("wq16", [128, HC * H * D], f16)
    wo16d = din("wo16", [128, H * HID], f16)
    cosK = din("cosK", [128, KVW], f16)
    sinK = din("sinK", [128, KVW], f16)
    npadQ = din("npadQ", [1, R], f32)
    outT = nc.dram_tensor("outT", [HC, 128, R], f16, kind="ExternalOutput").ap()

    with tile.TileContext(nc) as tc, ExitStack() as ctx:
        const = ctx.enter_context(tc.tile_pool(name="const", bufs=1))
        kvp = ctx.enter_context(tc.tile_pool(name="kvp", bufs=1))
        work = ctx.enter_context(tc.tile_pool(name="work", bufs=2))
        psG = ctx.enter_context(tc.tile_pool(name="psG", bufs=2, space="PSUM"))
        psC = ctx.enter_context(tc.tile_pool(name="psC", bufs=2, space="PSUM"))
        psM = ctx.enter_context(tc.tile_pool(name="psM", bufs=1, space="PSUM"))
        psD = ctx.enter_context(tc.tile_pool(name="psD", bufs=1, space="PSUM"))

        # ---------------- input loads (dependency-gated waves)
        wk_sb = const.tile([128, HC, KVH * D], f8)
        wv_sb = const.tile([128, HC, KVH * D], f8)
        wk16_sb = const.tile([128, HC, KVH * D], f16)
        wv16_sb = const.tile([128, HC, KVH * D], f16)
        wq_sb = const.tile([128, HC, H * D], f16)
        wo_sb = const.tile([128, H, HID], f16)
        cosK_sb = const.tile([128, KVW], f16)
        sinK_sb = const.tile([128, KVW], f16)
        npad_sb = const.tile([1, R], f32)
        xT_sb = const.tile([128, HC, W], f8)
        xT16_sb = const.tile([128, HC, R], f16)

        def xblk_ap(i):
            return bass.AP(tensor=xT8.tensor, offset=i * 128 * HC * 512,
                           ap=[[HC * 512, 128], [512, HC], [1, 512]])

        def gated(gate, out, in_):
            di = nc.sync.dma_start(out=out, in_=in_)
            if gate is not None:
                tile.add_dep_helper(di.ins, gate.ins, sync=True,
                                    reason="dma priority wave")
            return di

        nc.sync.dma_start(out=wk_sb, in_=wk8)
        nc.sync.dma_start(out=wv_sb, in_=wv8)
        g0 = nc.sync.dma_start(out=xT_sb[:, :, 0:512], in_=xblk_ap(0))
        gated(g0, cosK_sb, cosK)
        gated(g0, sinK_sb, sinK)
        g1 = gated(g0, xT_sb[:, :, 512:1024], xblk_ap(1))
        g2 = gated(g1, xT16_sb, xT16d)
        gated(g1, wk16_sb, wk16d)
        gated(g1, wv16_sb, wv16d)
        gated(g1, npad_sb, npadQ)
        g3 = gated(g2, xT_sb[:, :, 1024:1536], xblk_ap(2))
        g3 = gated(g2, xT_sb[:, :, 1536:2048], xblk_ap(3))
        g4 = gated(g3, wq_sb, wq16d)
        gated(g4, wo_sb, wo16d)

        ones8 = const.tile([128, 2, 16], f8)
        nc.vector.memset(ones8, 1.0)
        ones16c = const.tile([128, 1], f16)
        nc.vector.memset(ones16c, 1.0)
        ones_row16 = const.tile([1, 128], f16)
        nc.vector.memset(ones_row16, 1.0)
        # upper-edge mask (valid iff query_local >= key_partition), f16 0/1
        mask_up = const.tile([128, R], f16)
        nc.gpsimd.memset(mask_up, 1.0)
        nc.gpsimd.affine_select(
            out=mask_up, in_=mask_up, compare_op=mybir.AluOpType.is_ge,
            fill=0.0, base=0, pattern=[[1, R]], channel_multiplier=-1)

        # ---------------- RoPE ([128, width] f32 psum -> f16 sbuf, DVE only)
        def rope(dst, src_ps, cos_ap, sin_ap, width):
            sb = work.tile([128, 512], f16, tag="ropesrc")
            nc.scalar.copy(out=sb[:, :width], in_=src_ps)
            tmp = work.tile([128, 512], f16, tag="rtmp")
            nc.vector.tensor_copy(out=tmp[0:64, :width], in_=sb[64:128, :width])
            nc.vector.tensor_copy(out=tmp[64:128, :width], in_=sb[0:64, :width])
            ta = work.tile([128, 512], f16, tag="ra")
            nc.vector.tensor_mul(ta[:, :width], sb[:, :width], cos_ap)
            tb = work.tile([128, 512], f16, tag="rb2")
            nc.vector.tensor_mul(tb[:, :width], tmp[:, :width], sin_ap)
            return nc.vector.tensor_add(dst, ta[:, :width], tb[:, :width])

        # ---------------- K/V projections + RoPE
        kT_sb = [kvp.tile([128, KVW], f16, name=f"kT{g}") for g in range(KVH)]
        v8_sb = kvp.tile([128, HIE, KVH * D], f8, name="v8")
        v16_sb = kvp.tile([128, NKC - HIE, KVH * D], f16, name="v16")

        krot = [0]

        def kps_tile(bi):
            k = krot[0] % 3
            krot[0] += 1
            if k < 2:
                t = psG.tile([128, 1024], f32, tag="sc", name=f"kps{bi}")
                return t[:, :512]
            return psM.tile([128, 512], f32, tag="mm", name=f"kpsm{bi}")

        vrot = [0]

        def vps_tile(kc):
            k = vrot[0] % 3
            vrot[0] += 1
            if k < 2:
                return psC.tile([128, 512], f32, tag="ctx", name=f"vps{kc}")
            return psD.tile([128, 512], f32, tag="den", name=f"vpsd{kc}")

        def emit_kv_block8(bi):      # fp8 DoubleRow blocks 0-3
            b0 = bi * 512
            for g in range(KVH):
                kps = kps_tile(bi)
                for a in range(NPAIR):
                    nc.tensor.matmul(
                        kps,
                        lhsT=wk_sb[:, 2 * a:2 * a + 2, g * D:(g + 1) * D],
                        rhs=xT_sb[:, 2 * a:2 * a + 2, b0:b0 + 512],
                        start=(a == 0), stop=(a == NPAIR - 1), perf_mode=DR)
                rope(kT_sb[g][:, b0:b0 + 512], kps,
                     cosK_sb[:, b0:b0 + 512], sinK_sb[:, b0:b0 + 512], 512)
            for j in range(4):
                kc = bi * 4 + j
                vps = vps_tile(kc)[:, :256]
                for a in range(NPAIR):
                    nc.tensor.matmul(
                        vps,
                        lhsT=xT_sb[:, 2 * a:2 * a + 2, kc * 128:(kc + 1) * 128],
                        rhs=wv_sb[:, 2 * a:2 * a + 2, :],
                        start=(a == 0), stop=(a == NPAIR - 1), perf_mode=DR)
                nc.vector.tensor_scalar_mul(v8_sb[:, kc, :], vps, 1.0 / WS)

        def emit_kv_block16():       # f16 block 4 (own rows)
            b0 = W
            for g in range(KVH):
                kps = kps_tile(4)
                for c in range(HC):
                    nc.tensor.matmul(
                        kps, lhsT=wk16_sb[:, c, g * D:(g + 1) * D],
                        rhs=xT16_sb[:, c, :],
                        start=(c == 0), stop=(c == HC - 1))
                rope(kT_sb[g][:, b0:b0 + 512], kps,
                     cosK_sb[:, b0:b0 + 512], sinK_sb[:, b0:b0 + 512], 512)
            for j in range(4):
                vps = vps_tile(HIE + j)[:, :256]
                for c in range(HC):
                    nc.tensor.matmul(
                        vps, lhsT=xT16_sb[:, c, j * 128:(j + 1) * 128],
                        rhs=wv16_sb[:, c, :],
                        start=(c == 0), stop=(c == HC - 1))
                nc.vector.tensor_copy(out=v16_sb[:, j, :], in_=vps)

        # ---------------- attention
        # chunk-pair groups: (a,b) with a fp8-path chunk and b f16-path chunk
        # for mixed groups; pure fp8 pairs use one DoubleRow matmul.
        dr_pairs = [(4, 5), (6, 7), (8, 9), (10, 11), (12, 13), (14, 15)]
        order = [(4, 5), (6, 7), (3, 16), (8, 9), (2, 17), (10, 11),
                 (1, 18), (12, 13), (0, 19), (14, 15)]
        NG = len(order)

        ctxn = kvp.tile([128, H, R], f16, name="ctxn")
        qTs = {}

        def emit_q(h):
            qps = psM.tile([128, 512], f32, tag="mm")
            for c in range(HC):
                nc.tensor.matmul(
                    qps[:, :R], lhsT=wq_sb[:, c, h * D:(h + 1) * D],
                    rhs=xT16_sb[:, c, :],
                    start=(c == 0), stop=(c == HC - 1))
            qT = work.tile([128, R], f16, tag="qT")
            rope(qT, qps[:, :R], cosK_sb[:, W:KVW], sinK_sb[:, W:KVW], R)
            qTs[h] = qT

        def widths(a, b):            # compacted widths for a mixed pair
            wa = min(R, (a + 1) * 128)
            wb = R - (b - HIE) * 128
            assert wa == wb
            return wa

        def emit_scores(h, gi, qT):
            g = h // GROUPS
            a, b = order[gi]
            scps = psG.tile([128, 1024], f32, tag="sc")
            if (a, b) in dr_pairs:
                nc.tensor.matmul(scps[:, 0:512], lhsT=kT_sb[g][:, ts(a, 128)],
                                 rhs=qT, start=True, stop=True)
                nc.tensor.matmul(scps[:, 512:1024],
                                 lhsT=kT_sb[g][:, ts(b, 128)],
                                 rhs=qT, start=True, stop=True)
            else:
                wa = widths(a, b)
                nc.tensor.matmul(scps[:, 0:wa], lhsT=kT_sb[g][:, ts(a, 128)],
                                 rhs=qT[:, 0:wa], start=True, stop=True)
                nc.tensor.matmul(scps[:, 512:512 + wa],
                                 lhsT=kT_sb[g][:, ts(b, 128)],
                                 rhs=qT[:, R - wa:R], start=True, stop=True)
            return scps

        def emit_exp_mask(gi, scps):
            a, b = order[gi]
            if (a, b) in dr_pairs:
                P8 = work.tile([128, 1024], f8, tag="P", bufs=3)
                nc.scalar.activation(out=P8, in_=scps, func=Exp, scale=ESC_LO)
                return P8, None
            wa = widths(a, b)
            P8 = work.tile([128, 512], f8, tag="P8m", bufs=3)
            nc.scalar.activation(out=P8[:, 0:wa], in_=scps[:, 0:wa],
                                 func=Exp, scale=ESC_LO)
            P16 = work.tile([128, 512], f16, tag="P16", bufs=3)
            nc.scalar.activation(out=P16[:, 0:wa], in_=scps[:, 512:512 + wa],
                                 func=Exp, scale=ESC_HI)
            # lower-edge mask on fp8 plane (gpsimd), upper-edge on f16 (DVE)
            nc.gpsimd.affine_select(
                out=P8[:, 0:wa], in_=P8[:, 0:wa],
                compare_op=mybir.AluOpType.is_ge, fill=0.0,
                base=128 * a - 1, pattern=[[-1, wa]], channel_multiplier=1)
            nc.vector.tensor_mul(P16[:, 0:wa], P16[:, 0:wa], mask_up[:, 0:wa])
            return P8, P16

        def emit_pv_den(h, gi, P8, P16, ctx_ps, den_ps):
            g = h // GROUPS
            a, b = order[gi]
            first = gi == 0
            last = gi == NG - 1
            if (a, b) in dr_pairs:
                prhs = bass.AP(tensor=P8.tensor, offset=P8.offset,
                               ap=[list(P8.ap[0]), [512, 2], [1, 512]])
                nc.tensor.matmul(ctx_ps,
                                 lhsT=v8_sb[:, a:a + 2, g * D:(g + 1) * D],
                                 rhs=prhs, start=first, stop=last,
                                 skip_group_check=True, perf_mode=DR)
                nc.tensor.matmul(den_ps, lhsT=ones8[:, :, 0:1], rhs=prhs,
                                 start=first, stop=last,
                                 skip_group_check=True, perf_mode=DR)
            else:
                wa = widths(a, b)
                nc.tensor.matmul(ctx_ps[:, R - wa:R],
                                 lhsT=v16_sb[:, b - HIE, g * D:(g + 1) * D],
                                 rhs=P16[:, 0:wa], start=False, stop=False,
                                 skip_group_check=True)
                nc.tensor.matmul(den_ps[:, R - wa:R], lhsT=ones16c,
                                 rhs=P16[:, 0:wa], start=False, stop=False,
                                 skip_group_check=True)
                nc.tensor.matmul(ctx_ps[:, 0:wa],
                                 lhsT=v8_sb[:, a, g * D:(g + 1) * D],
                                 rhs=P8[:, 0:wa], start=False, stop=False,
                                 skip_group_check=True)
                nc.tensor.matmul(den_ps[:, 0:wa], lhsT=ones8[:, 0, 0:1],
                                 rhs=P8[:, 0:wa], start=False, stop=False,
                                 skip_group_check=True)

        def emit_finish(h, ctx_ps, den_ps):
            drow = work.tile([1, R], f32, tag="drow")
            nc.vector.tensor_sub(drow, den_ps, npad_sb)
            rrow = work.tile([1, R], f32, tag="rrow")
            nc.vector.reciprocal_approx_fast(out=rrow, in_=drow)
            rrow16 = work.tile([1, R], f16, tag="rrow16")
            nc.vector.tensor_copy(out=rrow16, in_=rrow)
            bc_ps = psM.tile([128, 512], f32, tag="mm")
            nc.tensor.matmul(bc_ps[:, :R], lhsT=ones_row16, rhs=rrow16,
                             start=True, stop=True)
            rbc = work.tile([128, R], f16, tag="rbc")
            nc.vector.tensor_copy(out=rbc, in_=bc_ps[:, :R])
            nc.vector.tensor_mul(ctxn[:, h, :], ctx_ps, rbc)

        def emit_attn(h, finish_prev):
            qT = qTs.pop(h)
            ctx_ps = psC.tile([128, R], f32, tag="ctx")
            den_ps = psD.tile([1, R], f32, tag="den")
            scs = [emit_scores(h, 0, qT), emit_scores(h, 1, qT)]
            for gi in range(NG):
                P8, P16 = emit_exp_mask(gi, scs[gi])
                if gi + 2 < NG:
                    scs.append(emit_scores(h, gi + 2, qT))
                if gi == 4 and h + 1 < H:
                    emit_q(h + 1)
                emit_pv_den(h, gi, P8, P16, ctx_ps, den_ps)
                if gi == 0 and finish_prev is not None:
                    finish_prev()
            return lambda: emit_finish(h, ctx_ps, den_ps)

        emit_kv_block8(0)
        emit_kv_block8(1)
        emit_kv_block16()
        emit_kv_block8(2)
        emit_kv_block8(3)
        emit_q(0)
        fin = None
        for h in range(H):
            fin = emit_attn(h, fin)
        fin()

        # ---------------- o_proj (f16)
        for ot in range(HC):
            ops = psC.tile([128, R], f32, tag="ctx")
            for hh in range(H):
                nc.tensor.matmul(ops, lhsT=wo_sb[:, hh, ts(ot, 128)],
                                 rhs=ctxn[:, hh, :],
                                 start=(hh == 0), stop=(hh == H - 1))
            ob = work.tile([128, R], f16, tag="ob")
            nc.scalar.copy(out=ob, in_=ops)
            nc.sync.dma_start(out=outT[ot], in_=ob)

    nc.compile()
    return nc


# ---------------------------------------------------------------- host side
def host_prep(cfg, x, wq, wk, wv, wo, pos):
    """x: [S, HID] f32, weights as in reference, pos: [S] int. Returns list of
    per-core input dicts."""
    from concourse import mybir

    f8np = mybir.dt.np(mybir.dt.float8e4)
    R, W, HID, H, KVH, D, TH = (cfg["R"], cfg["W"], cfg["HID"], cfg["H"],
                                cfg["KVH"], cfg["D"], cfg["THETA"])
    WS = cfg["WSCALE"]
    KVW, HC, NKC = _derived(cfg)
    S = x.shape[0]
    ncores = S // R
    inv_freq = (1.0 / TH ** (np.arange(0, D, 2, dtype=np.float64) / D))

    def pack_pm(wt, ncol, scale, dt):
        a = (wt * scale).reshape(-1, 128, ncol)
        return np.ascontiguousarray(
            a.transpose(1, 0, 2).reshape(128, -1)).astype(dt)

    wk8 = pack_pm(wk.T, KVH * D, WS, f8np)
    wv8 = pack_pm(wv.T, KVH * D, WS, f8np)
    wk16 = pack_pm(wk.T, KVH * D, 1.0, np.float16)
    wv16 = pack_pm(wv.T, KVH * D, 1.0, np.float16)
    wq16 = pack_pm(wq.T, H * D, 1.0, np.float16)
    wo16 = pack_pm(wo.T, HID, 1.0, np.float16)

    in_maps = []
    for c in range(ncores):
        lo, hi = c * R - W, c * R + R
        pad = max(0, -lo)
        xw = np.zeros((KVW, HID), np.float32)
        xw[pad:] = x[max(lo, 0):hi]
        xTa = xw.T.reshape(HC, 128, KVW)                  # [c, p, j]
        parts = []
        for b0 in range(0, W, 512):
            blk = xTa[:, :, b0:b0 + 512].transpose(1, 0, 2)
            parts.append(np.ascontiguousarray(blk).reshape(-1))
        xT8 = np.concatenate(parts).astype(f8np)
        xT16 = np.ascontiguousarray(
            xTa[:, :, W:KVW].transpose(1, 0, 2)).astype(np.float16)

        pw = np.zeros(KVW, np.float64)
        pw[pad:] = pos[max(lo, 0):hi].astype(np.float64)
        ang = pw[:, None] * inv_freq[None, :]             # [KVW, 64]
        ck, sk = np.cos(ang).T, np.sin(ang).T             # [64, KVW]
        cosK = np.concatenate([ck, ck], 0).astype(np.float16)
        sinK = np.concatenate([-sk, sk], 0).astype(np.float16)
        i_idx = np.arange(R, dtype=np.float32)
        npad = np.maximum(0.0, pad - 1.0 - i_idx)[None, :].astype(np.float32)

        in_maps.append(dict(xT8=xT8, xT16=xT16, wk8=wk8, wv8=wv8,
                            wk16=wk16, wv16=wv16, wq16=wq16, wo16=wo16,
                            cosK=cosK, sinK=sinK, npadQ=npad))
    return in_maps


def assemble(cfg, outs):
    """outs: list of per-core outT arrays [HC, 128, R] f16 -> [S, HID] f32."""
    R, HID = cfg["R"], cfg["HID"]
    blocks = [o.astype(np.float32).transpose(2, 0, 1).reshape(R, HID)
              for o in outs]
    return np.concatenate(blocks, 0)


_PROGRAM_CACHE = {}


def kernel(hidden_states, wq, wk, wv, wo, position_ids):
    from concourse.bass_utils import run_bass_kernel_spmd

    cfg = FULL_CFG
    x = np.asarray(hidden_states, np.float32)
    assert x.ndim == 3 and x.shape[0] == 1
    x2 = x[0]
    pos = np.asarray(position_ids)[0]
    in_maps = host_prep(cfg, x2, np.asarray(wq, np.float32),
                        np.asarray(wk, np.float32), np.asarray(wv, np.float32),
                        np.asarray(wo, np.float32), pos)
    key = "full"
    if key not in _PROGRAM_CACHE:
        _PROGRAM_CACHE[key] = build_program(cfg)
    nc = _PROGRAM_CACHE[key]
    res = run_bass_kernel_spmd(nc, in_maps, list(range(cfg["NCORES"])))
    outs = [res.results[i]["outT"] for i in range(cfg["NCORES"])]
    out = assemble(cfg, outs)
    return out.reshape(1, *out.shape).astype(np.float32)


# revision 29
# speedup vs baseline: 1.0437x; 1.0437x over previous
"""Trainium2 Bass kernel for Mistral sliding-window attention (B=1, S=4096,
HID=1024, H=8 q-heads, KVH=2 kv-heads, D=128, WINDOW=2048).

Sequence-sharded across 8 NeuronCores (512 queries per core); each core
recomputes K/V for its 2560-key window.

Precision plan: fp8(e4m3) DoubleRow matmuls for the bulk work — K/V
projections of the 2048-key history (blocks 0-3) and P@V / denominator for
key chunks 0-15 — and f16 for the numerically sensitive edge: the core's own
512 rows (block 4: K/V/Q projections), P for key chunks 16-19, ctx, and
o_proj. Core 0's earliest queries attend only chunk-16+ keys (tiny softmax
windows, no error averaging), so their whole path stays f16; all other
regions average fp8 noise over >=512 keys.

wk8/wv8 are pre-scaled by 64 on the host (fp8 subnormal avoidance); the
device rescales V by 1/64 when writing v8 and folds the K-side 64 into the
exp scale of chunks 0-15.
"""

import sys
import numpy as np
from contextlib import ExitStack

if "/opt/trn_rl_repo" not in sys.path:
    sys.path.insert(0, "/opt/trn_rl_repo")

# ---------------------------------------------------------------- constants
FULL_CFG = dict(
    R=512,        # queries per core
    W=2048,       # sliding window
    HID=1024,     # hidden size
    H=8,          # query heads
    KVH=2,        # kv heads
    D=128,        # head dim
    THETA=10000.0,
    NCORES=8,
    WSCALE=64.0,  # host fp8 weight pre-scale
)


def _derived(cfg):
    R, W, HID = cfg["R"], cfg["W"], cfg["HID"]
    KVW = W + R
    HC = HID // 128
    NKC = KVW // 128
    assert W >= R and R <= 512 and HID % 128 == 0 and KVW % 128 == 0
    return KVW, HC, NKC


def build_program(cfg):
    import concourse.bass as bass
    import concourse.tile as tile
    from concourse import bacc, mybir

    f32, f16, f8 = mybir.dt.float32, mybir.dt.float16, mybir.dt.float8e4
    DR = mybir.MatmulPerfMode.DoubleRow
    Exp = mybir.ActivationFunctionType.Exp
    ts = bass.ts
    R, W, HID, H, KVH, D = (cfg["R"], cfg["W"], cfg["HID"], cfg["H"],
                            cfg["KVH"], cfg["D"])
    WS = cfg["WSCALE"]
    KVW, HC, NKC = _derived(cfg)
    GROUPS = H // KVH
    NPAIR = HC // 2
    HIE = W // 128              # first upper-edge (own-rows) chunk = 16
    ESC_LO = 1.0 / (np.sqrt(D) * WS)    # fp8 chunks: k carries x64
    ESC_HI = 1.0 / np.sqrt(D)           # f16 chunks

    nc = bacc.Bacc("TRN2", target_bir_lowering=False, debug=False)

    def din(name, shape, dt):
        return nc.dram_tensor(name, shape, dt, kind="ExternalInput").ap()

    xT8 = din("xT8", [128 * HC * W], f8)          # blocks 0-3, block-major
    xT16d = din("xT16", [128, HC, R], f16)        # block 4 (own rows)
    wk8 = din("wk8", [128, HC * KVH * D], f8)
    wv8 = din("wv8", [128, HC * KVH * D], f8)
    wk16d = din("wk16", [128, HC * KVH * D], f16)
    wv16d = din("wv16", [128, HC * KVH * D], f16)
    wq16d = din("wq16", [128, HC * H * D], f16)
    wo16d = din("wo16", [128, H * HID], f16)
    cosK = din("cosK", [128, KVW], f16)
    sinK = din("sinK", [128, KVW], f16)
    npadQ = din("npadQ", [1, R], f32)
    outT = nc.dram_tensor("outT", [HC, 128, R], f16, kind="ExternalOutput").ap()

    with tile.TileContext(nc) as tc, ExitStack() as ctx:
        const = ctx.enter_context(tc.tile_pool(name="const", bufs=1))
        kvp = ctx.enter_context(tc.tile_pool(name="kvp", bufs=1))
        work = ctx.enter_context(tc.tile_pool(name="work", bufs=2))
        psG = ctx.enter_context(tc.tile_pool(name="psG", bufs=2, space="PSUM"))
        psC = ctx.enter_context(tc.tile_pool(name="psC", bufs=2, space="PSUM"))
        psM = ctx.enter_context(tc.tile_pool(name="psM", bufs=1, space="PSUM"))
        psD = ctx.enter_context(tc.tile_pool(name="psD", bufs=1, space="PSUM"))

        # ---------------- input loads (dependency-gated waves)
        wk_sb = const.tile([128, HC, KVH * D], f8)
        wv_sb = const.tile([128, HC, KVH * D], f8)
        wk16_sb = const.tile([128, HC, KVH * D], f16)
        wv16_sb = const.tile([128, HC, KVH * D], f16)
        wq_sb = const.tile([128, HC, H * D], f16)
        wo_sb = const.tile([128, H, HID], f16)
        cosK_sb = const.tile([128, KVW], f16)
        sinK_sb = const.tile([128, KVW], f16)
        npad_sb = const.tile([1, R], f32)
        xT_sb = const.tile([128, HC, W], f8)
        xT16_sb = const.tile([128, HC, R], f16)

        def xblk_ap(i):
            return bass.AP(tensor=xT8.tensor, offset=i * 128 * HC * 512,
                           ap=[[HC * 512, 128], [512, HC], [1, 512]])

        def gated(gate, out, in_):
            di = nc.sync.dma_start(out=out, in_=in_)
            if gate is not None:
                tile.add_dep_helper(di.ins, gate.ins, sync=True,
                                    reason="dma priority wave")
            return di

        nc.sync.dma_start(out=wk_sb, in_=wk8)
        nc.sync.dma_start(out=wv_sb, in_=wv8)
        g0 = nc.sync.dma_start(out=xT_sb[:, :, 0:512], in_=xblk_ap(0))
        gated(g0, cosK_sb, cosK)
        gated(g0, sinK_sb, sinK)
        g1 = gated(g0, xT_sb[:, :, 512:1024], xblk_ap(1))
        g2 = gated(g1, xT16_sb, xT16d)
        gated(g1, wk16_sb, wk16d)
        gated(g1, wv16_sb, wv16d)
        gated(g1, npad_sb, npadQ)
        g3 = gated(g2, xT_sb[:, :, 1024:1536], xblk_ap(2))
        g3 = gated(g2, xT_sb[:, :, 1536:2048], xblk_ap(3))
        g4 = gated(g3, wq_sb, wq16d)
        gated(g4, wo_sb, wo16d)

        ones8 = const.tile([128, 2, 16], f8)
        nc.vector.memset(ones8, 1.0)
        ones16c = const.tile([128, 1], f16)
        nc.vector.memset(ones16c, 1.0)
        ones_row16 = const.tile([1, 128], f16)
        nc.vector.memset(ones_row16, 1.0)
        # upper-edge mask (valid iff query_local >= key_partition), f16 0/1
        mask_up = const.tile([128, R], f16)
        nc.gpsimd.memset(mask_up, 1.0)
        nc.gpsimd.affine_select(
            out=mask_up, in_=mask_up, compare_op=mybir.AluOpType.is_ge,
            fill=0.0, base=0, pattern=[[1, R]], channel_multiplier=-1)

        # ---------------- RoPE ([128, width] f32 psum -> f16 sbuf).
        # use_act=True routes the psum-freeing first copy to ScalarE — only
        # safe for K ropes (KV phase, ACT idle). Q-prefetch ropes run during
        # attention where ACT owns the exp chain, so they stay on DVE.
        def rope(dst, src_ps, cos_ap, sin_ap, width, use_act=False):
            sb = work.tile([128, 512], f16, tag="ropesrc")
            if use_act:
                nc.scalar.copy(out=sb[:, :width], in_=src_ps)
            else:
                nc.vector.tensor_copy(out=sb[:, :width], in_=src_ps)
            tmp = work.tile([128, 512], f16, tag="rtmp")
            nc.vector.tensor_copy(out=tmp[0:64, :width], in_=sb[64:128, :width])
            nc.vector.tensor_copy(out=tmp[64:128, :width], in_=sb[0:64, :width])
            ta = work.tile([128, 512], f16, tag="ra")
            nc.vector.tensor_mul(ta[:, :width], sb[:, :width], cos_ap)
            tb = work.tile([128, 512], f16, tag="rb2")
            nc.vector.tensor_mul(tb[:, :width], tmp[:, :width], sin_ap)
            return nc.vector.tensor_add(dst, ta[:, :width], tb[:, :width])

        # ---------------- K/V projections + RoPE
        kT_sb = [kvp.tile([128, KVW], f16, name=f"kT{g}") for g in range(KVH)]
        v8_sb = kvp.tile([128, HIE, KVH * D], f8, name="v8")
        v16_sb = kvp.tile([128, NKC - HIE, KVH * D], f16, name="v16")

        krot = [0]

        def kps_tile(bi):
            k = krot[0] % 3
            krot[0] += 1
            if k < 2:
                t = psG.tile([128, 1024], f32, tag="sc", name=f"kps{bi}")
                return t[:, :512]
            return psM.tile([128, 512], f32, tag="mm", name=f"kpsm{bi}")

        vrot = [0]

        def vps_tile(kc):
            k = vrot[0] % 3
            vrot[0] += 1
            if k < 2:
                return psC.tile([128, 512], f32, tag="ctx", name=f"vps{kc}")
            return psD.tile([128, 512], f32, tag="den", name=f"vpsd{kc}")

        def emit_kv_block8(bi):      # fp8 DoubleRow blocks 0-3
            b0 = bi * 512
            for g in range(KVH):
                kps = kps_tile(bi)
                for a in range(NPAIR):
                    nc.tensor.matmul(
                        kps,
                        lhsT=wk_sb[:, 2 * a:2 * a + 2, g * D:(g + 1) * D],
                        rhs=xT_sb[:, 2 * a:2 * a + 2, b0:b0 + 512],
                        start=(a == 0), stop=(a == NPAIR - 1), perf_mode=DR)
                rope(kT_sb[g][:, b0:b0 + 512], kps,
                     cosK_sb[:, b0:b0 + 512], sinK_sb[:, b0:b0 + 512], 512,
                     use_act=True)
            for j in range(4):
                kc = bi * 4 + j
                vps = vps_tile(kc)[:, :256]
                for a in range(NPAIR):
                    nc.tensor.matmul(
                        vps,
                        lhsT=xT_sb[:, 2 * a:2 * a + 2, kc * 128:(kc + 1) * 128],
                        rhs=wv_sb[:, 2 * a:2 * a + 2, :],
                        start=(a == 0), stop=(a == NPAIR - 1), perf_mode=DR)
                nc.vector.tensor_scalar_mul(v8_sb[:, kc, :], vps, 1.0 / WS)

        def emit_kv_block16():       # f16 block 4 (own rows)
            b0 = W
            for g in range(KVH):
                kps = kps_tile(4)
                for c in range(HC):
                    nc.tensor.matmul(
                        kps, lhsT=wk16_sb[:, c, g * D:(g + 1) * D],
                        rhs=xT16_sb[:, c, :],
                        start=(c == 0), stop=(c == HC - 1))
                rope(kT_sb[g][:, b0:b0 + 512], kps,
                     cosK_sb[:, b0:b0 + 512], sinK_sb[:, b0:b0 + 512], 512,
                     use_act=True)
            for j in range(4):
                vps = vps_tile(HIE + j)[:, :256]
                for c in range(HC):
                    nc.tensor.matmul(
                        vps, lhsT=xT16_sb[:, c, j * 128:(j + 1) * 128],
                        rhs=wv16_sb[:, c, :],
                        start=(c == 0), stop=(c == HC - 1))
                nc.vector.tensor_copy(out=v16_sb[:, j, :], in_=vps)

        # ---------------- attention
        # chunk-pair groups: (a,b) with a fp8-path chunk and b f16-path chunk
        # for mixed groups; pure fp8 pairs use one DoubleRow matmul.
        dr_pairs = [(4, 5), (6, 7), (8, 9), (10, 11), (12, 13), (14, 15)]
        order = [(4, 5), (6, 7), (3, 16), (8, 9), (2, 17), (10, 11),
                 (1, 18), (12, 13), (0, 19), (14, 15)]
        NG = len(order)

        ctxn = kvp.tile([128, H, R], f16, name="ctxn")
        qTs = {}

        def emit_q(h):
            qps = psM.tile([128, 512], f32, tag="mm")
            for c in range(HC):
                nc.tensor.matmul(
                    qps[:, :R], lhsT=wq_sb[:, c, h * D:(h + 1) * D],
                    rhs=xT16_sb[:, c, :],
                    start=(c == 0), stop=(c == HC - 1))
            qT = work.tile([128, R], f16, tag="qT")
            rope(qT, qps[:, :R], cosK_sb[:, W:KVW], sinK_sb[:, W:KVW], R)
            qTs[h] = qT

        def widths(a, b):            # compacted widths for a mixed pair
            wa = min(R, (a + 1) * 128)
            wb = R - (b - HIE) * 128
            assert wa == wb
            return wa

        def emit_scores(h, gi, qT):
            g = h // GROUPS
            a, b = order[gi]
            scps = psG.tile([128, 1024], f32, tag="sc")
            if (a, b) in dr_pairs:
                nc.tensor.matmul(scps[:, 0:512], lhsT=kT_sb[g][:, ts(a, 128)],
                                 rhs=qT, start=True, stop=True)
                nc.tensor.matmul(scps[:, 512:1024],
                                 lhsT=kT_sb[g][:, ts(b, 128)],
                                 rhs=qT, start=True, stop=True)
            else:
                wa = widths(a, b)
                nc.tensor.matmul(scps[:, 0:wa], lhsT=kT_sb[g][:, ts(a, 128)],
                                 rhs=qT[:, 0:wa], start=True, stop=True)
                nc.tensor.matmul(scps[:, 512:512 + wa],
                                 lhsT=kT_sb[g][:, ts(b, 128)],
                                 rhs=qT[:, R - wa:R], start=True, stop=True)
            return scps

        def emit_exp_mask(gi, scps):
            a, b = order[gi]
            if (a, b) in dr_pairs:
                P8 = work.tile([128, 1024], f8, tag="P", bufs=3)
                nc.scalar.activation(out=P8, in_=scps, func=Exp, scale=ESC_LO)
                return P8, None
            wa = widths(a, b)
            P8 = work.tile([128, 512], f8, tag="P8m", bufs=3)
            nc.scalar.activation(out=P8[:, 0:wa], in_=scps[:, 0:wa],
                                 func=Exp, scale=ESC_LO)
            P16 = work.tile([128, 512], f16, tag="P16", bufs=3)
            nc.scalar.activation(out=P16[:, 0:wa], in_=scps[:, 512:512 + wa],
                                 func=Exp, scale=ESC_HI)
            # lower-edge mask on fp8 plane (gpsimd), upper-edge on f16 (DVE)
            nc.gpsimd.affine_select(
                out=P8[:, 0:wa], in_=P8[:, 0:wa],
                compare_op=mybir.AluOpType.is_ge, fill=0.0,
                base=128 * a - 1, pattern=[[-1, wa]], channel_multiplier=1)
            nc.vector.tensor_mul(P16[:, 0:wa], P16[:, 0:wa], mask_up[:, 0:wa])
            return P8, P16

        def emit_pv_den(h, gi, P8, P16, ctx_ps, den_ps):
            g = h // GROUPS
            a, b = order[gi]
            first = gi == 0
            last = gi == NG - 1
            if (a, b) in dr_pairs:
                prhs = bass.AP(tensor=P8.tensor, offset=P8.offset,
                               ap=[list(P8.ap[0]), [512, 2], [1, 512]])
                nc.tensor.matmul(ctx_ps,
                                 lhsT=v8_sb[:, a:a + 2, g * D:(g + 1) * D],
                                 rhs=prhs, start=first, stop=last,
                                 skip_group_check=True, perf_mode=DR)
                nc.tensor.matmul(den_ps, lhsT=ones8[:, :, 0:1], rhs=prhs,
                                 start=first, stop=last,
                                 skip_group_check=True, perf_mode=DR)
            else:
                wa = widths(a, b)
                nc.tensor.matmul(ctx_ps[:, 0:wa],
                                 lhsT=v8_sb[:, a, g * D:(g + 1) * D],
                                 rhs=P8[:, 0:wa], start=False, stop=False,
                                 skip_group_check=True)
                nc.tensor.matmul(den_ps[:, 0:wa], lhsT=ones8[:, 0, 0:1],
                                 rhs=P8[:, 0:wa], start=False, stop=False,
                                 skip_group_check=True)
                nc.tensor.matmul(ctx_ps[:, R - wa:R],
                                 lhsT=v16_sb[:, b - HIE, g * D:(g + 1) * D],
                                 rhs=P16[:, 0:wa], start=False, stop=False,
                                 skip_group_check=True)
                nc.tensor.matmul(den_ps[:, R - wa:R], lhsT=ones16c,
                                 rhs=P16[:, 0:wa], start=False, stop=False,
                                 skip_group_check=True)

        def emit_finish(h, ctx_ps, den_ps):
            drow = work.tile([1, R], f32, tag="drow")
            nc.vector.tensor_sub(drow, den_ps, npad_sb)
            rrow = work.tile([1, R], f32, tag="rrow")
            nc.vector.reciprocal_approx_fast(out=rrow, in_=drow)
            rrow16 = work.tile([1, R], f16, tag="rrow16")
            nc.vector.tensor_copy(out=rrow16, in_=rrow)
            bc_ps = psM.tile([128, 512], f32, tag="mm")
            nc.tensor.matmul(bc_ps[:, :R], lhsT=ones_row16, rhs=rrow16,
                             start=True, stop=True)
            rbc = work.tile([128, R], f16, tag="rbc")
            nc.vector.tensor_copy(out=rbc, in_=bc_ps[:, :R])
            nc.vector.tensor_mul(ctxn[:, h, :], ctx_ps, rbc)

        def emit_attn(h, finish_prev):
            qT = qTs.pop(h)
            ctx_ps = psC.tile([128, R], f32, tag="ctx")
            den_ps = psD.tile([1, R], f32, tag="den")
            scs = [emit_scores(h, 0, qT), emit_scores(h, 1, qT)]
            for gi in range(NG):
                P8, P16 = emit_exp_mask(gi, scs[gi])
                if gi + 2 < NG:
                    scs.append(emit_scores(h, gi + 2, qT))
                if gi == 4 and h + 1 < H:
                    emit_q(h + 1)
                emit_pv_den(h, gi, P8, P16, ctx_ps, den_ps)
                if gi == 0 and finish_prev is not None:
                    finish_prev()
            return lambda: emit_finish(h, ctx_ps, den_ps)

        emit_kv_block8(0)
        emit_kv_block8(1)
        emit_kv_block16()
        emit_kv_block8(2)
        emit_kv_block8(3)
        emit_q(0)
        fin = None
        for h in range(H):
            fin = emit_attn(h, fin)
        fin()

        # ---------------- o_proj (f16)
        for ot in range(HC):
            ops = psC.tile([128, R], f32, tag="ctx")
            for hh in range(H):
                nc.tensor.matmul(ops, lhsT=wo_sb[:, hh, ts(ot, 128)],
                                 rhs=ctxn[:, hh, :],
                                 start=(hh == 0), stop=(hh == H - 1))
            ob = work.tile([128, R], f16, tag="ob")
            nc.scalar.copy(out=ob, in_=ops)
            nc.sync.dma_start(out=outT[ot], in_=ob)

    nc.compile()
    return nc


# ---------------------------------------------------------------- host side
def host_prep(cfg, x, wq, wk, wv, wo, pos):
    """x: [S, HID] f32, weights as in reference, pos: [S] int. Returns list of
    per-core input dicts."""
    from concourse import mybir

    f8np = mybir.dt.np(mybir.dt.float8e4)
    R, W, HID, H, KVH, D, TH = (cfg["R"], cfg["W"], cfg["HID"], cfg["H"],
                                cfg["KVH"], cfg["D"], cfg["THETA"])
    WS = cfg["WSCALE"]
    KVW, HC, NKC = _derived(cfg)
    S = x.shape[0]
    ncores = S // R
    inv_freq = (1.0 / TH ** (np.arange(0, D, 2, dtype=np.float64) / D))

    def pack_pm(wt, ncol, scale, dt):
        a = (wt * scale).reshape(-1, 128, ncol)
        return np.ascontiguousarray(
            a.transpose(1, 0, 2).reshape(128, -1)).astype(dt)

    wk8 = pack_pm(wk.T, KVH * D, WS, f8np)
    wv8 = pack_pm(wv.T, KVH * D, WS, f8np)
    wk16 = pack_pm(wk.T, KVH * D, 1.0, np.float16)
    wv16 = pack_pm(wv.T, KVH * D, 1.0, np.float16)
    wq16 = pack_pm(wq.T, H * D, 1.0, np.float16)
    wo16 = pack_pm(wo.T, HID, 1.0, np.float16)

    in_maps = []
    for c in range(ncores):
        lo, hi = c * R - W, c * R + R
        pad = max(0, -lo)
        xw = np.zeros((KVW, HID), np.float32)
        xw[pad:] = x[max(lo, 0):hi]
        xTa = xw.T.reshape(HC, 128, KVW)                  # [c, p, j]
        parts = []
        for b0 in range(0, W, 512):
            blk = xTa[:, :, b0:b0 + 512].transpose(1, 0, 2)
            parts.append(np.ascontiguousarray(blk).reshape(-1))
        xT8 = np.concatenate(parts).astype(f8np)
        xT16 = np.ascontiguousarray(
            xTa[:, :, W:KVW].transpose(1, 0, 2)).astype(np.float16)

        pw = np.zeros(KVW, np.float64)
        pw[pad:] = pos[max(lo, 0):hi].astype(np.float64)
        ang = pw[:, None] * inv_freq[None, :]             # [KVW, 64]
        ck, sk = np.cos(ang).T, np.sin(ang).T             # [64, KVW]
        cosK = np.concatenate([ck, ck], 0).astype(np.float16)
        sinK = np.concatenate([-sk, sk], 0).astype(np.float16)
        i_idx = np.arange(R, dtype=np.float32)
        npad = np.maximum(0.0, pad - 1.0 - i_idx)[None, :].astype(np.float32)

        in_maps.append(dict(xT8=xT8, xT16=xT16, wk8=wk8, wv8=wv8,
                            wk16=wk16, wv16=wv16, wq16=wq16, wo16=wo16,
                            cosK=cosK, sinK=sinK, npadQ=npad))
    return in_maps


def assemble(cfg, outs):
    """outs: list of per-core outT arrays [HC, 128, R] f16 -> [S, HID] f32."""
    R, HID = cfg["R"], cfg["HID"]
    blocks = [o.astype(np.float32).transpose(2, 0, 1).reshape(R, HID)
              for o in outs]
    return np.concatenate(blocks, 0)


_PROGRAM_CACHE = {}


def kernel(hidden_states, wq, wk, wv, wo, position_ids):
    from concourse.bass_utils import run_bass_kernel_spmd

    cfg = FULL_CFG
    x = np.asarray(hidden_states, np.float32)
    assert x.ndim == 3 and x.shape[0] == 1
    x2 = x[0]
    pos = np.asarray(position_ids)[0]
    in_maps = host_prep(cfg, x2, np.asarray(wq, np.float32),
                        np.asarray(wk, np.float32), np.asarray(wv, np.float32),
                        np.asarray(wo, np.float32), pos)
    key = "full"
    if key not in _PROGRAM_CACHE:
        _PROGRAM_CACHE[key] = build_program(cfg)
    nc = _PROGRAM_CACHE[key]
    res = run_bass_kernel_spmd(nc, in_maps, list(range(cfg["NCORES"])))
    outs = [res.results[i]["outT"] for i in range(cfg["NCORES"])]
    out = assemble(cfg, outs)
    return out.reshape(1, *out.shape).astype(np.float32)


# revision 30
# speedup vs baseline: 1.0571x; 1.0128x over previous
"""Trainium2 Bass kernel for Mistral sliding-window attention (B=1, S=4096,
HID=1024, H=8 q-heads, KVH=2 kv-heads, D=128, WINDOW=2048).

Sequence-sharded across 8 NeuronCores (512 queries per core); each core
recomputes K/V for its 2560-key window.

Precision plan: fp8(e4m3) DoubleRow matmuls for the bulk work — K/V
projections of the 2048-key history (blocks 0-3) and P@V / denominator for
key chunks 0-15 — and f16 for the numerically sensitive edge: the core's own
512 rows (block 4: K/V/Q projections), P for key chunks 16-19, ctx, and
o_proj. Core 0's earliest queries attend only chunk-16+ keys (tiny softmax
windows, no error averaging), so their whole path stays f16; all other
regions average fp8 noise over >=512 keys.

wk8/wv8 are pre-scaled by 64 on the host (fp8 subnormal avoidance); the
device rescales V by 1/64 when writing v8 and folds the K-side 64 into the
exp scale of chunks 0-15.
"""

import sys
import numpy as np
from contextlib import ExitStack

if "/opt/trn_rl_repo" not in sys.path:
    sys.path.insert(0, "/opt/trn_rl_repo")

# ---------------------------------------------------------------- constants
FULL_CFG = dict(
    R=512,        # queries per core
    W=2048,       # sliding window
    HID=1024,     # hidden size
    H=8,          # query heads
    KVH=2,        # kv heads
    D=128,        # head dim
    THETA=10000.0,
    NCORES=8,
    WSCALE=64.0,  # host fp8 weight pre-scale
)


def _derived(cfg):
    R, W, HID = cfg["R"], cfg["W"], cfg["HID"]
    KVW = W + R
    HC = HID // 128
    NKC = KVW // 128
    assert W >= R and R <= 512 and HID % 128 == 0 and KVW % 128 == 0
    return KVW, HC, NKC


def build_program(cfg):
    import concourse.bass as bass
    import concourse.tile as tile
    from concourse import bacc, mybir

    f32, f16, f8 = mybir.dt.float32, mybir.dt.float16, mybir.dt.float8e4
    DR = mybir.MatmulPerfMode.DoubleRow
    Exp = mybir.ActivationFunctionType.Exp
    ts = bass.ts
    R, W, HID, H, KVH, D = (cfg["R"], cfg["W"], cfg["HID"], cfg["H"],
                            cfg["KVH"], cfg["D"])
    WS = cfg["WSCALE"]
    KVW, HC, NKC = _derived(cfg)
    GROUPS = H // KVH
    NPAIR = HC // 2
    HIE = W // 128              # first upper-edge (own-rows) chunk = 16
    ESC_LO = 1.0 / (np.sqrt(D) * WS)    # fp8 chunks: k carries x64
    ESC_HI = 1.0 / np.sqrt(D)           # f16 chunks

    nc = bacc.Bacc("TRN2", target_bir_lowering=False, debug=False)

    def din(name, shape, dt):
        return nc.dram_tensor(name, shape, dt, kind="ExternalInput").ap()

    xT8 = din("xT8", [128 * HC * W], f8)          # blocks 0-3, block-major
    xT16d = din("xT16", [128, HC, R], f16)        # block 4 (own rows)
    wk8 = din("wk8", [128, HC * KVH * D], f8)
    wv8 = din("wv8", [128, HC * KVH * D], f8)
    wk16d = din("wk16", [128, HC * KVH * D], f16)
    wv16d = din("wv16", [128, HC * KVH * D], f16)
    wq16d = din("wq16", [128, HC * H * D], f16)
    wo16d = din("wo16", [128, H * HID], f16)
    cosK = din("cosK", [128, KVW], f16)
    sinK = din("sinK", [128, KVW], f16)
    npadQ = din("npadQ", [1, R], f32)
    outT = nc.dram_tensor("outT", [HC, 128, R], f16, kind="ExternalOutput").ap()

    with tile.TileContext(nc) as tc, ExitStack() as ctx:
        const = ctx.enter_context(tc.tile_pool(name="const", bufs=1))
        kvp = ctx.enter_context(tc.tile_pool(name="kvp", bufs=1))
        work = ctx.enter_context(tc.tile_pool(name="work", bufs=2))
        psG = ctx.enter_context(tc.tile_pool(name="psG", bufs=2, space="PSUM"))
        psC = ctx.enter_context(tc.tile_pool(name="psC", bufs=2, space="PSUM"))
        psM = ctx.enter_context(tc.tile_pool(name="psM", bufs=1, space="PSUM"))
        psD = ctx.enter_context(tc.tile_pool(name="psD", bufs=1, space="PSUM"))

        # ---------------- input loads (dependency-gated waves)
        wk_sb = const.tile([128, HC, KVH * D], f8)
        wv_sb = const.tile([128, HC, KVH * D], f8)
        wk16_sb = const.tile([128, HC, KVH * D], f16)
        wv16_sb = const.tile([128, HC, KVH * D], f16)
        wq_sb = const.tile([128, HC, H * D], f16)
        wo_sb = const.tile([128, H, HID], f16)
        cosK_sb = const.tile([128, KVW], f16)
        sinK_sb = const.tile([128, KVW], f16)
        npad_sb = const.tile([1, R], f32)
        xT_sb = const.tile([128, HC, W], f8)
        xT16_sb = const.tile([128, HC, R], f16)

        def xblk_ap(i):
            return bass.AP(tensor=xT8.tensor, offset=i * 128 * HC * 512,
                           ap=[[HC * 512, 128], [512, HC], [1, 512]])

        def gated(gate, out, in_):
            di = nc.sync.dma_start(out=out, in_=in_)
            if gate is not None:
                tile.add_dep_helper(di.ins, gate.ins, sync=True,
                                    reason="dma priority wave")
            return di

        nc.sync.dma_start(out=wk_sb, in_=wk8)
        nc.sync.dma_start(out=wv_sb, in_=wv8)
        g0 = nc.sync.dma_start(out=xT_sb[:, :, 0:512], in_=xblk_ap(0))
        gated(g0, cosK_sb, cosK)
        gated(g0, sinK_sb, sinK)
        g1 = gated(g0, xT_sb[:, :, 512:1024], xblk_ap(1))
        g2 = gated(g1, xT16_sb, xT16d)
        gated(g1, wk16_sb, wk16d)
        gated(g1, wv16_sb, wv16d)
        gated(g1, npad_sb, npadQ)
        g3 = gated(g2, xT_sb[:, :, 1024:1536], xblk_ap(2))
        g3 = gated(g2, xT_sb[:, :, 1536:2048], xblk_ap(3))
        g4 = gated(g3, wq_sb, wq16d)
        gated(g4, wo_sb, wo16d)

        ones8 = const.tile([128, 2, 16], f8)
        nc.vector.memset(ones8, 1.0)
        ones16c = const.tile([128, 1], f16)
        nc.vector.memset(ones16c, 1.0)
        ones_row16 = const.tile([1, 128], f16)
        nc.vector.memset(ones_row16, 1.0)
        # upper-edge mask (valid iff query_local >= key_partition), f16 0/1
        mask_up = const.tile([128, R], f16)
        nc.gpsimd.memset(mask_up, 1.0)
        nc.gpsimd.affine_select(
            out=mask_up, in_=mask_up, compare_op=mybir.AluOpType.is_ge,
            fill=0.0, base=0, pattern=[[1, R]], channel_multiplier=-1)

        # ---------------- RoPE ([128, width] f32 psum -> f16 sbuf).
        # use_act=True routes the psum-freeing first copy to ScalarE — only
        # safe for K ropes (KV phase, ACT idle). Q-prefetch ropes run during
        # attention where ACT owns the exp chain, so they stay on DVE.
        def rope(dst, src_ps, cos_ap, sin_ap, width, use_act=False):
            sb = work.tile([128, 512], f16, tag="ropesrc")
            if use_act:
                nc.scalar.copy(out=sb[:, :width], in_=src_ps)
            else:
                nc.vector.tensor_copy(out=sb[:, :width], in_=src_ps)
            tmp = work.tile([128, 512], f16, tag="rtmp")
            if use_act:
                nc.scalar.copy(out=tmp[0:64, :width], in_=sb[64:128, :width])
                nc.scalar.copy(out=tmp[64:128, :width], in_=sb[0:64, :width])
            else:
                nc.vector.tensor_copy(out=tmp[0:64, :width],
                                      in_=sb[64:128, :width])
                nc.vector.tensor_copy(out=tmp[64:128, :width],
                                      in_=sb[0:64, :width])
            ta = work.tile([128, 512], f16, tag="ra")
            nc.vector.tensor_mul(ta[:, :width], sb[:, :width], cos_ap)
            tb = work.tile([128, 512], f16, tag="rb2")
            nc.vector.tensor_mul(tb[:, :width], tmp[:, :width], sin_ap)
            return nc.vector.tensor_add(dst, ta[:, :width], tb[:, :width])

        # ---------------- K/V projections + RoPE
        kT_sb = [kvp.tile([128, KVW], f16, name=f"kT{g}") for g in range(KVH)]
        v8_sb = kvp.tile([128, HIE, KVH * D], f8, name="v8")
        v16_sb = kvp.tile([128, NKC - HIE, KVH * D], f16, name="v16")

        krot = [0]

        def kps_tile(bi):
            k = krot[0] % 3
            krot[0] += 1
            if k < 2:
                t = psG.tile([128, 1024], f32, tag="sc", name=f"kps{bi}")
                return t[:, :512]
            return psM.tile([128, 512], f32, tag="mm", name=f"kpsm{bi}")

        vrot = [0]

        def vps_tile(kc):
            k = vrot[0] % 3
            vrot[0] += 1
            if k < 2:
                return psC.tile([128, 512], f32, tag="ctx", name=f"vps{kc}")
            return psD.tile([128, 512], f32, tag="den", name=f"vpsd{kc}")

        def emit_kv_block8(bi):      # fp8 DoubleRow blocks 0-3
            b0 = bi * 512
            for g in range(KVH):
                kps = kps_tile(bi)
                for a in range(NPAIR):
                    nc.tensor.matmul(
                        kps,
                        lhsT=wk_sb[:, 2 * a:2 * a + 2, g * D:(g + 1) * D],
                        rhs=xT_sb[:, 2 * a:2 * a + 2, b0:b0 + 512],
                        start=(a == 0), stop=(a == NPAIR - 1), perf_mode=DR)
                rope(kT_sb[g][:, b0:b0 + 512], kps,
                     cosK_sb[:, b0:b0 + 512], sinK_sb[:, b0:b0 + 512], 512,
                     use_act=True)
            for j in range(4):
                kc = bi * 4 + j
                vps = vps_tile(kc)[:, :256]
                for a in range(NPAIR):
                    nc.tensor.matmul(
                        vps,
                        lhsT=xT_sb[:, 2 * a:2 * a + 2, kc * 128:(kc + 1) * 128],
                        rhs=wv_sb[:, 2 * a:2 * a + 2, :],
                        start=(a == 0), stop=(a == NPAIR - 1), perf_mode=DR)
                nc.vector.tensor_scalar_mul(v8_sb[:, kc, :], vps, 1.0 / WS)

        def emit_kv_block16():       # f16 block 4 (own rows)
            b0 = W
            for g in range(KVH):
                kps = kps_tile(4)
                for c in range(HC):
                    nc.tensor.matmul(
                        kps, lhsT=wk16_sb[:, c, g * D:(g + 1) * D],
                        rhs=xT16_sb[:, c, :],
                        start=(c == 0), stop=(c == HC - 1))
                rope(kT_sb[g][:, b0:b0 + 512], kps,
                     cosK_sb[:, b0:b0 + 512], sinK_sb[:, b0:b0 + 512], 512,
                     use_act=True)
            for j in range(4):
                vps = vps_tile(HIE + j)[:, :256]
                for c in range(HC):
                    nc.tensor.matmul(
                        vps, lhsT=xT16_sb[:, c, j * 128:(j + 1) * 128],
                        rhs=wv16_sb[:, c, :],
                        start=(c == 0), stop=(c == HC - 1))
                nc.vector.tensor_copy(out=v16_sb[:, j, :], in_=vps)

        # ---------------- attention
        # chunk-pair groups: (a,b) with a fp8-path chunk and b f16-path chunk
        # for mixed groups; pure fp8 pairs use one DoubleRow matmul.
        dr_pairs = [(4, 5), (6, 7), (8, 9), (10, 11), (12, 13), (14, 15)]
        order = [(4, 5), (6, 7), (3, 16), (8, 9), (2, 17), (10, 11),
                 (1, 18), (12, 13), (0, 19), (14, 15)]
        NG = len(order)

        ctxn = kvp.tile([128, H, R], f16, name="ctxn")
        qTs = {}

        def emit_q(h):
            qps = psM.tile([128, 512], f32, tag="mm")
            for c in range(HC):
                nc.tensor.matmul(
                    qps[:, :R], lhsT=wq_sb[:, c, h * D:(h + 1) * D],
                    rhs=xT16_sb[:, c, :],
                    start=(c == 0), stop=(c == HC - 1))
            qT = work.tile([128, R], f16, tag="qT")
            rope(qT, qps[:, :R], cosK_sb[:, W:KVW], sinK_sb[:, W:KVW], R)
            qTs[h] = qT

        def widths(a, b):            # compacted widths for a mixed pair
            wa = min(R, (a + 1) * 128)
            wb = R - (b - HIE) * 128
            assert wa == wb
            return wa

        def emit_scores(h, gi, qT):
            g = h // GROUPS
            a, b = order[gi]
            scps = psG.tile([128, 1024], f32, tag="sc")
            if (a, b) in dr_pairs:
                nc.tensor.matmul(scps[:, 0:512], lhsT=kT_sb[g][:, ts(a, 128)],
                                 rhs=qT, start=True, stop=True)
                nc.tensor.matmul(scps[:, 512:1024],
                                 lhsT=kT_sb[g][:, ts(b, 128)],
                                 rhs=qT, start=True, stop=True)
            else:
                wa = widths(a, b)
                nc.tensor.matmul(scps[:, 0:wa], lhsT=kT_sb[g][:, ts(a, 128)],
                                 rhs=qT[:, 0:wa], start=True, stop=True)
                nc.tensor.matmul(scps[:, 512:512 + wa],
                                 lhsT=kT_sb[g][:, ts(b, 128)],
                                 rhs=qT[:, R - wa:R], start=True, stop=True)
            return scps

        def emit_exp_mask(gi, scps):
            a, b = order[gi]
            if (a, b) in dr_pairs:
                P8 = work.tile([128, 1024], f8, tag="P", bufs=3)
                nc.scalar.activation(out=P8, in_=scps, func=Exp, scale=ESC_LO)
                return P8, None
            wa = widths(a, b)
            P8 = work.tile([128, 512], f8, tag="P8m", bufs=3)
            nc.scalar.activation(out=P8[:, 0:wa], in_=scps[:, 0:wa],
                                 func=Exp, scale=ESC_LO)
            P16 = work.tile([128, 512], f16, tag="P16", bufs=3)
            nc.scalar.activation(out=P16[:, 0:wa], in_=scps[:, 512:512 + wa],
                                 func=Exp, scale=ESC_HI)
            # lower-edge mask on fp8 plane (gpsimd), upper-edge on f16 (DVE)
            nc.gpsimd.affine_select(
                out=P8[:, 0:wa], in_=P8[:, 0:wa],
                compare_op=mybir.AluOpType.is_ge, fill=0.0,
                base=128 * a - 1, pattern=[[-1, wa]], channel_multiplier=1)
            nc.vector.tensor_mul(P16[:, 0:wa], P16[:, 0:wa], mask_up[:, 0:wa])
            return P8, P16

        def emit_pv_den(h, gi, P8, P16, ctx_ps, den_ps):
            g = h // GROUPS
            a, b = order[gi]
            first = gi == 0
            last = gi == NG - 1
            if (a, b) in dr_pairs:
                prhs = bass.AP(tensor=P8.tensor, offset=P8.offset,
                               ap=[list(P8.ap[0]), [512, 2], [1, 512]])
                nc.tensor.matmul(ctx_ps,
                                 lhsT=v8_sb[:, a:a + 2, g * D:(g + 1) * D],
                                 rhs=prhs, start=first, stop=last,
                                 skip_group_check=True, perf_mode=DR)
                nc.tensor.matmul(den_ps, lhsT=ones8[:, :, 0:1], rhs=prhs,
                                 start=first, stop=last,
                                 skip_group_check=True, perf_mode=DR)
            else:
                wa = widths(a, b)
                nc.tensor.matmul(ctx_ps[:, 0:wa],
                                 lhsT=v8_sb[:, a, g * D:(g + 1) * D],
                                 rhs=P8[:, 0:wa], start=False, stop=False,
                                 skip_group_check=True)
                nc.tensor.matmul(den_ps[:, 0:wa], lhsT=ones8[:, 0, 0:1],
                                 rhs=P8[:, 0:wa], start=False, stop=False,
                                 skip_group_check=True)
                nc.tensor.matmul(ctx_ps[:, R - wa:R],
                                 lhsT=v16_sb[:, b - HIE, g * D:(g + 1) * D],
                                 rhs=P16[:, 0:wa], start=False, stop=False,
                                 skip_group_check=True)
                nc.tensor.matmul(den_ps[:, R - wa:R], lhsT=ones16c,
                                 rhs=P16[:, 0:wa], start=False, stop=False,
                                 skip_group_check=True)

        def emit_finish(h, ctx_ps, den_ps):
            drow = work.tile([1, R], f32, tag="drow")
            nc.vector.tensor_sub(drow, den_ps, npad_sb)
            rrow = work.tile([1, R], f32, tag="rrow")
            nc.vector.reciprocal_approx_fast(out=rrow, in_=drow)
            rrow16 = work.tile([1, R], f16, tag="rrow16")
            nc.vector.tensor_copy(out=rrow16, in_=rrow)
            bc_ps = psM.tile([128, 512], f32, tag="mm")
            nc.tensor.matmul(bc_ps[:, :R], lhsT=ones_row16, rhs=rrow16,
                             start=True, stop=True)
            rbc = work.tile([128, R], f16, tag="rbc")
            nc.vector.tensor_copy(out=rbc, in_=bc_ps[:, :R])
            nc.vector.tensor_mul(ctxn[:, h, :], ctx_ps, rbc)

        def emit_attn(h, finish_prev):
            qT = qTs.pop(h)
            ctx_ps = psC.tile([128, R], f32, tag="ctx")
            den_ps = psD.tile([1, R], f32, tag="den")
            scs = [emit_scores(h, 0, qT), emit_scores(h, 1, qT)]
            for gi in range(NG):
                P8, P16 = emit_exp_mask(gi, scs[gi])
                if gi + 2 < NG:
                    scs.append(emit_scores(h, gi + 2, qT))
                if gi == 4 and h + 1 < H:
                    emit_q(h + 1)
                emit_pv_den(h, gi, P8, P16, ctx_ps, den_ps)
                if gi == 0 and finish_prev is not None:
                    finish_prev()
            return lambda: emit_finish(h, ctx_ps, den_ps)

        emit_kv_block8(0)
        emit_kv_block8(1)
        emit_kv_block16()
        emit_kv_block8(2)
        emit_kv_block8(3)
        emit_q(0)
        fin = None
        for h in range(H):
            fin = emit_attn(h, fin)
        fin()

        # ---------------- o_proj (f16)
        for ot in range(HC):
            ops = psC.tile([128, R], f32, tag="ctx")
            for hh in range(H):
                nc.tensor.matmul(ops, lhsT=wo_sb[:, hh, ts(ot, 128)],
                                 rhs=ctxn[:, hh, :],
                                 start=(hh == 0), stop=(hh == H - 1))
            ob = work.tile([128, R], f16, tag="ob")
            nc.scalar.copy(out=ob, in_=ops)
            nc.sync.dma_start(out=outT[ot], in_=ob)

    nc.compile()
    return nc


# ---------------------------------------------------------------- host side
def host_prep(cfg, x, wq, wk, wv, wo, pos):
    """x: [S, HID] f32, weights as in reference, pos: [S] int. Returns list of
    per-core input dicts."""
    from concourse import mybir

    f8np = mybir.dt.np(mybir.dt.float8e4)
    R, W, HID, H, KVH, D, TH = (cfg["R"], cfg["W"], cfg["HID"], cfg["H"],
                                cfg["KVH"], cfg["D"], cfg["THETA"])
    WS = cfg["WSCALE"]
    KVW, HC, NKC = _derived(cfg)
    S = x.shape[0]
    ncores = S // R
    inv_freq = (1.0 / TH ** (np.arange(0, D, 2, dtype=np.float64) / D))

    def pack_pm(wt, ncol, scale, dt):
        a = (wt * scale).reshape(-1, 128, ncol)
        return np.ascontiguousarray(
            a.transpose(1, 0, 2).reshape(128, -1)).astype(dt)

    wk8 = pack_pm(wk.T, KVH * D, WS, f8np)
    wv8 = pack_pm(wv.T, KVH * D, WS, f8np)
    wk16 = pack_pm(wk.T, KVH * D, 1.0, np.float16)
    wv16 = pack_pm(wv.T, KVH * D, 1.0, np.float16)
    wq16 = pack_pm(wq.T, H * D, 1.0, np.float16)
    wo16 = pack_pm(wo.T, HID, 1.0, np.float16)

    in_maps = []
    for c in range(ncores):
        lo, hi = c * R - W, c * R + R
        pad = max(0, -lo)
        xw = np.zeros((KVW, HID), np.float32)
        xw[pad:] = x[max(lo, 0):hi]
        xTa = xw.T.reshape(HC, 128, KVW)                  # [c, p, j]
        parts = []
        for b0 in range(0, W, 512):
            blk = xTa[:, :, b0:b0 + 512].transpose(1, 0, 2)
            parts.append(np.ascontiguousarray(blk).reshape(-1))
        xT8 = np.concatenate(parts).astype(f8np)
        xT16 = np.ascontiguousarray(
            xTa[:, :, W:KVW].transpose(1, 0, 2)).astype(np.float16)

        pw = np.zeros(KVW, np.float64)
        pw[pad:] = pos[max(lo, 0):hi].astype(np.float64)
        ang = pw[:, None] * inv_freq[None, :]             # [KVW, 64]
        ck, sk = np.cos(ang).T, np.sin(ang).T             # [64, KVW]
        cosK = np.concatenate([ck, ck], 0).astype(np.float16)
        sinK = np.concatenate([-sk, sk], 0).astype(np.float16)
        i_idx = np.arange(R, dtype=np.float32)
        npad = np.maximum(0.0, pad - 1.0 - i_idx)[None, :].astype(np.float32)

        in_maps.append(dict(xT8=xT8, xT16=xT16, wk8=wk8, wv8=wv8,
                            wk16=wk16, wv16=wv16, wq16=wq16, wo16=wo16,
                            cosK=cosK, sinK=sinK, npadQ=npad))
    return in_maps


def assemble(cfg, outs):
    """outs: list of per-core outT arrays [HC, 128, R] f16 -> [S, HID] f32."""
    R, HID = cfg["R"], cfg["HID"]
    blocks = [o.astype(np.float32).transpose(2, 0, 1).reshape(R, HID)
              for o in outs]
    return np.concatenate(blocks, 0)


_PROGRAM_CACHE = {}


def kernel(hidden_states, wq, wk, wv, wo, position_ids):
    from concourse.bass_utils import run_bass_kernel_spmd

    cfg = FULL_CFG
    x = np.asarray(hidden_states, np.float32)
    assert x.ndim == 3 and x.shape[0] == 1
    x2 = x[0]
    pos = np.asarray(position_ids)[0]
    in_maps = host_prep(cfg, x2, np.asarray(wq, np.float32),
                        np.asarray(wk, np.float32), np.asarray(wv, np.float32),
                        np.asarray(wo, np.float32), pos)
    key = "full"
    if key not in _PROGRAM_CACHE:
        _PROGRAM_CACHE[key] = build_program(cfg)
    nc = _PROGRAM_CACHE[key]
    res = run_bass_kernel_spmd(nc, in_maps, list(range(cfg["NCORES"])))
    outs = [res.results[i]["outT"] for i in range(cfg["NCORES"])]
    out = assemble(cfg, outs)
    return out.reshape(1, *out.shape).astype(np.float32)
